# revision 1
# baseline (speedup 1.0000x reference)
"""GQA attention layer (B=2, S=2048, HID=4096, 32 Q heads / 8 KV heads, RoPE,
causal) on 8 TRN2 NeuronCores.

Strategy (tensor-parallel over heads):
  - core c owns Q heads 4c..4c+3 and KV head c (one full GQA group).
  - host pre-transposes x and the weight shards so every on-chip matmul
    contracts over the partition axis with no on-chip transposes of x.
  - bf16 matmuls throughout (f32 PSUM accumulation); projections emit
    feature-major Q^T/K^T/V^T with RoPE fused into the PSUM eviction;
    attention runs in S^T/k-major layout (exp on ACT, causal via
    multiplicative masks, softmax denominator accumulated on GpSimd +
    ones-matmul partition reduction).
  - ctx^T is AllGathered in 4 per-head-slot chunks (overlapped with
    attention of later heads); each core computes a 512-column slice of
    the o_proj output with host-permuted w_o rows matching the chunked
    gather layout; the host concatenates the slices.
"""

import os

import numpy as np
import ml_dtypes

B, S, HID = 2, 2048, 4096
NH, NKV, D = 32, 8, 128
T = B * S            # 4096 flattened tokens
NQ = 512             # per-core q features (4 heads x 128)
P = 128
TOKB = 512           # token block (matmul moving free dim)
NB = T // TOKB       # 8 token blocks
KC = HID // P        # 32 contraction chunks for projections
QBS = S // TOKB      # 4 q blocks per batch
KTS = S // P         # 16 k chunks per batch
NHL = 4              # local Q heads per core
SCALE = 1.0 / float(np.sqrt(np.float32(D)))
N_CORES = 8

_BUILT = None
LAST_RESULTS = None


def _build():
    from contextlib import ExitStack

    import concourse.tile as tile
    from concourse import bacc, mybir

    f32 = mybir.dt.float32
    bf16 = mybir.dt.bfloat16
    Exp = mybir.ActivationFunctionType.Exp

    nc = bacc.Bacc(
        "TRN2",
        target_bir_lowering=False,
        debug=False,
        num_devices=N_CORES,
    )

    xT = nc.declare_dram_parameter("xT", [HID, T], bf16, isOutput=False)
    wqT = nc.declare_dram_parameter("wqT", [HID, NQ], bf16, isOutput=False)
    wkT = nc.declare_dram_parameter("wkT", [HID, D], bf16, isOutput=False)
    wvT = nc.declare_dram_parameter("wvT", [HID, D], bf16, isOutput=False)
    woT = nc.declare_dram_parameter("woT", [HID, NQ], bf16, isOutput=False)
    cosT = nc.declare_dram_parameter("cosT", [64, T], f32, isOutput=False)
    sinT = nc.declare_dram_parameter("sinT", [64, T], f32, isOutput=False)
    maskT = nc.declare_dram_parameter("maskT", [P, 4 * TOKB], bf16, isOutput=False)
    outT = nc.declare_dram_parameter("outT", [NQ, T], f32, isOutput=True)

    with tile.TileContext(nc) as tc, ExitStack() as gctx:
        ec = gctx.enter_context
        # ---- global pools (whole-kernel lifetime) ----
        dram = ec(tc.tile_pool(name="dram", bufs=1, space="DRAM"))
        const_pool = ec(tc.tile_pool(name="const_sb", bufs=1))
        ps_pool = ec(tc.tile_pool(name="ps", bufs=3, space="PSUM"))
        ctxp_pool = ec(tc.tile_pool(name="ctxp", bufs=2, space="PSUM"))
        rs_pool = ec(tc.tile_pool(name="rsp", bufs=1, space="PSUM"))

        qt_dram = dram.tile([NQ, T], bf16)
        kt_dram = dram.tile([D, T], bf16)
        vt_dram = dram.tile([D, T], bf16)
        # token-chunked AllGather buffers: one per 512-token block, 0.5 MB
        # per rank -> mesh algorithm (RDH collectives hang intermittently)
        ag_in = [dram.tile([NQ, TOKB], bf16, name=f"ag_in{t}") for t in range(NB)]
        ag_out = [
            dram.tile([N_CORES * NQ, TOKB], bf16, addr_space="Shared",
                      name=f"ag_out{t}")
            for t in range(NB)
        ]

        ones_col = const_pool.tile([P, 1], f32, name="ones_col")
        nc.vector.memset(ones_col[:, :], 1.0)
        ones_row = const_pool.tile([1, P], f32, name="ones_row")
        nc.vector.memset(ones_row[:, :], 1.0)
        ident = const_pool.tile([P, P], bf16, name="ident")
        from concourse.masks import make_identity
        make_identity(nc, ident[:, :])

        # ================= Phase A: projections + RoPE =================
        with ExitStack() as actx:
            aec = actx.enter_context
            wq_pool = aec(tc.tile_pool(name="wq_sb", bufs=KC))
            wk_pool = aec(tc.tile_pool(name="wk_sb", bufs=KC))
            wv_pool = aec(tc.tile_pool(name="wv_sb", bufs=KC))
            xt_pool = aec(tc.tile_pool(name="xt_sb", bufs=KC + 8))
            rope_pool = aec(tc.tile_pool(name="rope_sb", bufs=1))
            evict_pool = aec(tc.tile_pool(name="evict_sb", bufs=4))
            rtmp_pool = aec(tc.tile_pool(name="rtmp_sb", bufs=4))

            cos_sb = rope_pool.tile([64, T], f32, name="cos_sb")
            nc.sync.dma_start(out=cos_sb[:, :], in_=cosT[:, :])
            sin_sb = rope_pool.tile([64, T], f32, name="sin_sb")
            nc.sync.dma_start(out=sin_sb[:, :], in_=sinT[:, :])

            wq_sb, wk_sb, wv_sb = [], [], []
            for kc in range(KC):
                wqt = wq_pool.tile([P, NQ], bf16, name="wq")
                nc.sync.dma_start(out=wqt[:, :], in_=wqT[P * kc:P * (kc + 1), :])
                wq_sb.append(wqt)
                wkt = wk_pool.tile([P, D], bf16, name="wk")
                nc.sync.dma_start(out=wkt[:, :], in_=wkT[P * kc:P * (kc + 1), :])
                wk_sb.append(wkt)
                wvt = wv_pool.tile([P, D], bf16, name="wv")
                nc.sync.dma_start(out=wvt[:, :], in_=wvT[P * kc:P * (kc + 1), :])
                wv_sb.append(wvt)

            def rope_evict(psum, dest_dram, rows, cols):
                """psum [128(d), 512(tok)] f32 -> RoPE -> SBUF bf16 -> DRAM."""
                ev = evict_pool.tile([P, TOKB], bf16, name="ev")
                c_ap = cos_sb[:, cols]
                s_ap = sin_sb[:, cols]
                p0 = psum[0:64, :]
                p1 = psum[64:128, :]
                t0 = rtmp_pool.tile([64, TOKB], f32, name="t0")
                t1 = rtmp_pool.tile([64, TOKB], f32, name="t1")
                nc.vector.tensor_mul(t0[:, :], p0, c_ap)
                nc.vector.tensor_mul(t1[:, :], p1, s_ap)
                nc.vector.tensor_sub(ev[0:64, :], t0[:, :], t1[:, :])
                t2 = rtmp_pool.tile([64, TOKB], f32, name="t2")
                t3 = rtmp_pool.tile([64, TOKB], f32, name="t3")
                nc.vector.tensor_mul(t2[:, :], p0, s_ap)
                nc.vector.tensor_mul(t3[:, :], p1, c_ap)
                nc.vector.tensor_add(ev[64:128, :], t2[:, :], t3[:, :])
                nc.sync.dma_start(out=dest_dram[rows, cols], in_=ev[:, :])

            for nb in range(NB):
                cols = slice(TOKB * nb, TOKB * (nb + 1))
                xts = []
                for kc in range(KC):
                    xt_t = xt_pool.tile([P, TOKB], bf16, name="xt")
                    nc.sync.dma_start(
                        out=xt_t[:, :], in_=xT[P * kc:P * (kc + 1), cols]
                    )
                    xts.append(xt_t)
                # Q heads
                for m in range(NHL):
                    psum = ps_pool.tile([P, TOKB], f32, name="ps")
                    for kc in range(KC):
                        nc.tensor.matmul(
                            psum[:, :],
                            wq_sb[kc][:, P * m:P * (m + 1)],
                            xts[kc][:, :],
                            start=(kc == 0),
                            stop=(kc == KC - 1),
                        )
                    rope_evict(psum, qt_dram, slice(P * m, P * (m + 1)), cols)
                # K
                psum = ps_pool.tile([P, TOKB], f32, name="ps")
                for kc in range(KC):
                    nc.tensor.matmul(
                        psum[:, :], wk_sb[kc][:, :], xts[kc][:, :],
                        start=(kc == 0), stop=(kc == KC - 1),
                    )
                rope_evict(psum, kt_dram, slice(0, D), cols)
                # V (no rope)
                psum = ps_pool.tile([P, TOKB], f32, name="ps")
                for kc in range(KC):
                    nc.tensor.matmul(
                        psum[:, :], wv_sb[kc][:, :], xts[kc][:, :],
                        start=(kc == 0), stop=(kc == KC - 1),
                    )
                ev = evict_pool.tile([P, TOKB], bf16, name="ev")
                nc.scalar.copy(ev[:, :], psum[:, :])
                nc.sync.dma_start(out=vt_dram[0:D, cols], in_=ev[:, :])

        # ================= Phase B: attention =================
        with ExitStack() as bctx:
            bec = bctx.enter_context
            mask_pool = bec(tc.tile_pool(name="mask_sb", bufs=1))
            kv_pool = bec(tc.tile_pool(name="kv_sb", bufs=B))
            vt_pool = bec(tc.tile_pool(name="vt_sb", bufs=B * KTS))
            qh_pool = bec(tc.tile_pool(name="qh_sb", bufs=4))
            e_pool = bec(tc.tile_pool(name="e_sb", bufs=6))
            acc_pool = bec(tc.tile_pool(name="acc_sb", bufs=3))
            norm_pool = bec(tc.tile_pool(name="norm_sb", bufs=2))
            ctx_out_pool = bec(tc.tile_pool(name="ctx_sb", bufs=4))

            mask_sb = mask_pool.tile([P, 4 * TOKB], bf16, name="mask_sb")
            nc.sync.dma_start(out=mask_sb[:, :], in_=maskT[:, :])

            # K^T resident per batch; V^T loaded then TensorE-transposed
            ktb = []
            v_sb = [[None] * KTS for _ in range(B)]
            for b in range(B):
                kt_t = kv_pool.tile([P, S], bf16, name="ktb")
                nc.sync.dma_start(
                    out=kt_t[:, :], in_=kt_dram[:, S * b:S * (b + 1)]
                )
                ktb.append(kt_t)
                vt_t = kv_pool.tile([P, S], bf16, name="vtb")
                nc.sync.dma_start(
                    out=vt_t[:, :], in_=vt_dram[:, S * b:S * (b + 1)]
                )
                for kt in range(KTS):
                    vps = rs_pool.tile([P, P], bf16, name="vps")
                    nc.tensor.transpose(
                        vps[:, :], vt_t[:, P * kt:P * (kt + 1)], ident[:, :]
                    )
                    vsb = vt_pool.tile([P, P], bf16, name="vsb")
                    nc.scalar.copy(vsb[:, :], vps[:, :])
                    v_sb[b][kt] = vsb

            for b in range(B):
                for qb in range(QBS):
                    tb = b * QBS + qb
                    for h in range(NHL):
                        qcols = slice(S * b + TOKB * qb, S * b + TOKB * (qb + 1))
                        qh = qh_pool.tile([P, TOKB], bf16, name="qh")
                        nc.sync.dma_start(
                            out=qh[:, :],
                            in_=qt_dram[P * h:P * (h + 1), qcols],
                        )
                        nkt = 4 * qb + 4
                        acc = acc_pool.tile([P, TOKB], f32, name="acc")
                        ctxp = ctxp_pool.tile([P, TOKB], f32, name="ctxp")
                        for kt in range(nkt):
                            sp = ps_pool.tile([P, TOKB], f32, name="ps")
                            nc.tensor.matmul(
                                sp[:, :],
                                ktb[b][:, P * kt:P * (kt + 1)],
                                qh[:, :],
                                start=True, stop=True,
                            )
                            e = e_pool.tile([P, TOKB], bf16, name="e")
                            nc.scalar.activation(e[:, :], sp[:, :], Exp, scale=SCALE)
                            j = kt - 4 * qb
                            if j >= 0:
                                nc.vector.tensor_mul(
                                    e[:, :], e[:, :],
                                    mask_sb[:, TOKB * j:TOKB * (j + 1)],
                                )
                            if kt == 0:
                                nc.vector.tensor_copy(acc[:, :], e[:, :])
                            else:
                                nc.vector.tensor_add(acc[:, :], acc[:, :], e[:, :])
                            nc.tensor.matmul(
                                ctxp[:, :],
                                v_sb[b][kt][:, :],
                                e[:, :],
                                start=(kt == 0), stop=(kt == nkt - 1),
                            )
                        # denominator: ones-matmul partition reduction (f32)
                        rs = rs_pool.tile([1, TOKB], f32, name="rs")
                        nc.tensor.matmul(
                            rs[:, :], ones_col[:, :], acc[:, :],
                            start=True, stop=True,
                        )
                        rcp = norm_pool.tile([1, TOKB], f32, name="rcp")
                        nc.vector.reciprocal(rcp[:, :], rs[:, :])
                        rbp = rs_pool.tile([P, TOKB], f32, name="rbp")
                        nc.tensor.matmul(
                            rbp[:, :], ones_row[:, :], rcp[:, :],
                            start=True, stop=True,
                        )
                        rbc = norm_pool.tile([P, TOKB], f32, name="rbc")
                        nc.scalar.copy(rbc[:, :], rbp[:, :])
                        ctx_sb = ctx_out_pool.tile([P, TOKB], bf16, name="ctx_sb")
                        nc.vector.tensor_mul(ctx_sb[:, :], ctxp[:, :], rbc[:, :])
                        nc.sync.dma_start(
                            out=ag_in[tb][P * h:P * (h + 1), :], in_=ctx_sb[:, :]
                        )
                    # gather this token block as soon as all 4 heads done
                    nc.gpsimd.collective_compute(
                        "AllGather",
                        mybir.AluOpType.bypass,
                        replica_groups=[list(range(N_CORES))],
                        ins=[ag_in[tb][:, :].opt()],
                        outs=[ag_out[tb][:, :].opt()],
                    )

        # ================= Phase D: o_proj slice =================
        # ag_out[tb] rows: 512c + 128h + d == 128*(4c+h) + d — the natural
        # global-head feature order, so woT needs no permutation.
        with ExitStack() as dctx:
            dec = dctx.enter_context
            wo_pool = dec(tc.tile_pool(name="wo_sb", bufs=KC))
            cx_pool = dec(tc.tile_pool(name="cx_sb", bufs=KC + 8))
            o_out_pool = dec(tc.tile_pool(name="o_sb", bufs=4))

            wo_sb = []
            for fc in range(KC):
                wot = wo_pool.tile([P, NQ], bf16, name="wo")
                nc.sync.dma_start(out=wot[:, :], in_=woT[P * fc:P * (fc + 1), :])
                wo_sb.append(wot)
            for tb in range(NB):
                cols = slice(TOKB * tb, TOKB * (tb + 1))
                cxs = []
                for fc in range(KC):
                    cx = cx_pool.tile([P, TOKB], bf16, name="cx")
                    nc.sync.dma_start(
                        out=cx[:, :],
                        in_=ag_out[tb][P * fc:P * (fc + 1), :],
                    )
                    cxs.append(cx)
                for ot in range(4):
                    psum = ps_pool.tile([P, TOKB], f32, name="ps")
                    for fc in range(KC):
                        nc.tensor.matmul(
                            psum[:, :],
                            wo_sb[fc][:, P * ot:P * (ot + 1)],
                            cxs[fc][:, :],
                            start=(fc == 0), stop=(fc == KC - 1),
                        )
                    ov = o_out_pool.tile([P, TOKB], f32, name="ov")
                    nc.scalar.copy(ov[:, :], psum[:, :])
                    nc.sync.dma_start(
                        out=outT[P * ot:P * (ot + 1), cols], in_=ov[:, :]
                    )

    nc.finalize()
    return nc


def _host_prep(x, positions, w_q, w_k, w_v, w_o):
    bf = ml_dtypes.bfloat16
    xT = np.ascontiguousarray(x.reshape(T, HID).T).astype(bf)

    half = D // 2
    inv_freq = 1.0 / (10000.0 ** (np.arange(half, dtype=np.float32) / half))
    freqs = np.outer(np.asarray(positions, np.float32), inv_freq)  # [S, 64]
    cosT1 = np.cos(freqs).T.astype(np.float32)  # [64, S]
    sinT1 = np.sin(freqs).T.astype(np.float32)
    cosT = np.ascontiguousarray(np.concatenate([cosT1] * B, axis=1))
    sinT = np.ascontiguousarray(np.concatenate([sinT1] * B, axis=1))

    dk = np.arange(P, dtype=np.int64)[:, None]
    dq = np.arange(TOKB, dtype=np.int64)[None, :]
    maskT = np.concatenate(
        [((dk + P * j) <= dq).astype(np.float32) for j in range(4)], axis=1
    ).astype(bf)
    maskT = np.ascontiguousarray(maskT)

    in_maps = []
    for c in range(N_CORES):
        wqTc = np.ascontiguousarray(w_q[NQ * c:NQ * (c + 1), :].T).astype(bf)
        wkTc = np.ascontiguousarray(w_k[D * c:D * (c + 1), :].T).astype(bf)
        wvTc = np.ascontiguousarray(w_v[D * c:D * (c + 1), :].T).astype(bf)
        woTc = np.ascontiguousarray(w_o[NQ * c:NQ * (c + 1), :].T).astype(bf)
        in_maps.append({
            "xT": xT, "wqT": wqTc, "wkT": wkTc, "wvT": wvTc, "woT": woTc,
            "cosT": cosT, "sinT": sinT, "maskT": maskT,
        })
    return in_maps


def _ensure_ntff_hook():
    """The agent image's antenv lacks axon_hooks; synthesize it so
    run_bass_kernel_spmd(trace=True) can capture NTFF profiles."""
    import sys
    import types
    try:
        from antenv.axon_hooks import get_axon_ntff_profile_hook  # noqa: F401
        return
    except ImportError:
        pass
    import antenv
    mod = types.ModuleType("antenv.axon_hooks")
    _h = [None]
    mod.set_axon_ntff_profile_hook = lambda h: _h.__setitem__(0, h)
    mod.get_axon_ntff_profile_hook = lambda: _h[0]
    sys.modules["antenv.axon_hooks"] = mod
    antenv.axon_hooks = mod
    try:
        from trn_agent_boot.trn_boot import _ntff_profile_via_ctypes
        mod.set_axon_ntff_profile_hook(
            _ntff_profile_via_ctypes("/opt/axon/libaxon_pjrt.so")
        )
    except Exception:
        pass


def kernel(x, positions, w_q, w_k, w_v, w_o):
    global _BUILT, LAST_RESULTS
    from concourse.bass_utils import run_bass_kernel_spmd

    x = np.asarray(x)
    positions = np.asarray(positions)
    w_q = np.asarray(w_q, np.float32)
    w_k = np.asarray(w_k, np.float32)
    w_v = np.asarray(w_v, np.float32)
    w_o = np.asarray(w_o, np.float32)

    if _BUILT is None:
        _BUILT = _build()
    nc = _BUILT

    in_maps = _host_prep(x, positions, w_q, w_k, w_v, w_o)
    trace = os.environ.get("BASS_KERNEL_TRACE", "0") == "1"
    if trace:
        _ensure_ntff_hook()
    res = run_bass_kernel_spmd(
        nc, in_maps, core_ids=list(range(N_CORES)), trace=trace
    )
    LAST_RESULTS = res

    out = np.empty((T, HID), np.float32)
    for c in range(N_CORES):
        out[:, NQ * c:NQ * (c + 1)] = np.asarray(res.results[c]["outT"]).T
    return out.reshape(B, S, HID)



# revision 12
# speedup vs baseline: 1.1955x; 1.1955x over previous
"""GQA attention layer (B=2, S=2048, HID=4096, 32 Q heads / 8 KV heads, RoPE,
causal) on 8 TRN2 NeuronCores.

Strategy (tensor-parallel over heads):
  - core c owns Q heads 4c..4c+3 and KV head c (one full GQA group).
  - host pre-transposes x and weights into [128, kc, feat] layouts so every
    input loads with a handful of large DMAs and every on-chip matmul
    contracts over the partition axis with no on-chip transposes of x.
  - Q/K/V are SBUF-resident: projections write Q^T/K^T (RoPE fused into the
    PSUM eviction on DVE) and V (TensorE-transposed) straight into
    persistent SBUF tiles; attention reads them with zero DMA.
  - attention is emitted score-lookahead (s0,s1,c0,s2,c1,...) so the PE
    FIFO never waits on the ACT exp; causal diagonal 128-chunks use
    narrowed q-ranges (exact triangle FLOPs) with a single 128x128
    triangular mask multiply; softmax denominator = one ones[128x128]
    matmul broadcast + DVE reciprocal.
  - ctx^T is AllGathered per 512-token block (8 chunks, non-RDH algo);
    o_proj for block tb-2 is interleaved between attention heads of block
    tb so the PE stream stays dense end-to-end; host concatenates the
    8 per-core 512-column output slices.
"""

import os

os.environ.setdefault("NEURON_RT_DBG_RDH_CC", "0")

import numpy as np
import ml_dtypes

B, S, HID = 2, 2048, 4096
NH, NKV, D = 32, 8, 128
T = B * S            # 4096 flattened tokens
NQ = 512             # per-core q features (4 heads x 128)
P = 128
TOKB = 512           # token block (matmul moving free dim)
NB = T // TOKB       # 8 token blocks
KC = HID // P        # 32 contraction chunks for projections
QBS = S // TOKB      # 4 q blocks per batch
KTS = S // P         # 16 k chunks per batch
NHL = 4              # local Q heads per core
SCALE = 1.0 / float(np.sqrt(np.float32(D)))
N_CORES = 8

_BUILT = None
LAST_RESULTS = None


def _build():
    from contextlib import ExitStack

    import concourse.tile as tile
    from concourse import bacc, mybir

    f32 = mybir.dt.float32
    bf16 = mybir.dt.bfloat16
    Exp = mybir.ActivationFunctionType.Exp

    nc = bacc.Bacc(
        "TRN2",
        target_bir_lowering=False,
        debug=False,
        num_devices=N_CORES,
    )

    xT = nc.declare_dram_parameter("xT", [P, KC, T], bf16, isOutput=False)
    wqT = nc.declare_dram_parameter("wqT", [P, KC, NQ], bf16, isOutput=False)
    wkT = nc.declare_dram_parameter("wkT", [P, KC, D], bf16, isOutput=False)
    wvT = nc.declare_dram_parameter("wvT", [P, KC, D], bf16, isOutput=False)
    woT = nc.declare_dram_parameter("woT", [P, KC, NQ], bf16, isOutput=False)
    cosT = nc.declare_dram_parameter("cosT", [64, T], f32, isOutput=False)
    sinT = nc.declare_dram_parameter("sinT", [64, T], f32, isOutput=False)
    maskT = nc.declare_dram_parameter("maskT", [P, P], bf16, isOutput=False)
    outT = nc.declare_dram_parameter("outT", [NQ, T], f32, isOutput=True)

    XC = 8               # kc chunks per x DMA
    NXD = KC // XC       # 4 x DMAs per token block

    with tile.TileContext(nc) as tc, ExitStack() as gctx:
        ec = gctx.enter_context
        # ---- global pools (whole-kernel lifetime) ----
        dram = ec(tc.tile_pool(name="dram", bufs=1, space="DRAM"))
        const_pool = ec(tc.tile_pool(name="const_sb", bufs=1))
        qkv_pool = ec(tc.tile_pool(name="qkv_sb", bufs=1))
        # PSUM budget (8 banks): ps 3 + ctxp 2 + rsp {vps,rbp} 2 + opp 1
        ps_pool = ec(tc.tile_pool(name="ps", bufs=3, space="PSUM"))
        ctxp_pool = ec(tc.tile_pool(name="ctxp", bufs=2, space="PSUM"))
        rs_pool = ec(tc.tile_pool(name="rsp", bufs=1, space="PSUM"))
        op_pool = ec(tc.tile_pool(name="opp", bufs=1, space="PSUM"))

        # token-chunked AllGather buffers (0.5 MB per rank per chunk)
        ag_in = [dram.tile([P, NHL * TOKB], bf16, name=f"ag_in{t}")
                 for t in range(NB)]
        ag_out = [
            dram.tile([N_CORES * P, NHL * TOKB], bf16, addr_space="Shared",
                      name=f"ag_out{t}")
            for t in range(NB)
        ]

        ones_sq = const_pool.tile([P, P], bf16, name="ones_sq")
        nc.vector.memset(ones_sq[:, :], 1.0)
        ident = const_pool.tile([P, P], bf16, name="ident")
        from concourse.masks import make_identity
        make_identity(nc, ident[:, :])
        tri_sb = const_pool.tile([P, P], bf16, name="tri_sb")
        nc.sync.dma_start(out=tri_sb[:, :], in_=maskT[:, :])

        # persistent Q^T / K^T / V tiles (SBUF-resident between phases)
        q_sb = qkv_pool.tile([P, NHL, T], bf16, name="q_sb")
        kt_sb = qkv_pool.tile([P, T], bf16, name="kt_sb")
        v_sb = [qkv_pool.tile([P, P], bf16, name=f"v_sb{g}")
                for g in range(T // P)]  # 32 chunks

        # ================= Phase A: projections + RoPE =================
        with ExitStack() as actx:
            aec = actx.enter_context
            w_pool = aec(tc.tile_pool(name="w_sb", bufs=1))
            xt_pool = aec(tc.tile_pool(name="xt_sb", bufs=5))
            rope_pool = aec(tc.tile_pool(name="rope_sb", bufs=1))
            rtmp_pool = aec(tc.tile_pool(name="rtmp_sb", bufs=2))
            vtmp_pool = aec(tc.tile_pool(name="vtmp_sb", bufs=2))

            wq_sb = w_pool.tile([P, KC, NQ], bf16, name="wq")
            nc.sync.dma_start(out=wq_sb[:, :, :], in_=wqT[:, :, :])
            wk_sb = w_pool.tile([P, KC, D], bf16, name="wk")
            nc.sync.dma_start(out=wk_sb[:, :, :], in_=wkT[:, :, :])
            wv_sb = w_pool.tile([P, KC, D], bf16, name="wv")
            nc.sync.dma_start(out=wv_sb[:, :, :], in_=wvT[:, :, :])
            cos_sb = rope_pool.tile([64, T], f32, name="cos_sb")
            nc.sync.dma_start(out=cos_sb[:, :], in_=cosT[:, :])
            sin_sb = rope_pool.tile([64, T], f32, name="sin_sb")
            nc.sync.dma_start(out=sin_sb[:, :], in_=sinT[:, :])

            def rope_evict(psum, dest, cols):
                """psum [128(d), 512(tok)] f32 -> RoPE -> dest bf16 slices."""
                c_ap = cos_sb[:, cols]
                s_ap = sin_sb[:, cols]
                p0 = psum[0:64, :]
                p1 = psum[64:128, :]
                t0 = rtmp_pool.tile([64, TOKB], f32, name="t0")
                t1 = rtmp_pool.tile([64, TOKB], f32, name="t1")
                nc.vector.tensor_mul(t0[:, :], p0, c_ap)
                nc.vector.tensor_mul(t1[:, :], p1, s_ap)
                nc.vector.tensor_sub(dest[0], t0[:, :], t1[:, :])
                t2 = rtmp_pool.tile([64, TOKB], f32, name="t2")
                t3 = rtmp_pool.tile([64, TOKB], f32, name="t3")
                nc.vector.tensor_mul(t2[:, :], p0, s_ap)
                nc.vector.tensor_mul(t3[:, :], p1, c_ap)
                nc.vector.tensor_add(dest[1], t2[:, :], t3[:, :])

            for nb in range(NB):
                cols = slice(TOKB * nb, TOKB * (nb + 1))
                xts = []
                for xd in range(NXD):
                    xt_t = xt_pool.tile([P, XC, TOKB], bf16, name="xt")
                    nc.sync.dma_start(
                        out=xt_t[:, :, :],
                        in_=xT[:, XC * xd:XC * (xd + 1), cols],
                    )
                    xts.append(xt_t)

                def xr(kc):
                    return xts[kc // XC][:, kc % XC, :]

                # Q heads
                for m in range(NHL):
                    psum = ps_pool.tile([P, TOKB], f32, name="ps")
                    for kc in range(KC):
                        nc.tensor.matmul(
                            psum[:, :],
                            wq_sb[:, kc, P * m:P * (m + 1)],
                            xr(kc),
                            start=(kc == 0),
                            stop=(kc == KC - 1),
                        )
                    rope_evict(
                        psum,
                        (q_sb[0:64, m, cols], q_sb[64:128, m, cols]),
                        cols,
                    )
                # K
                psum = ps_pool.tile([P, TOKB], f32, name="ps")
                for kc in range(KC):
                    nc.tensor.matmul(
                        psum[:, :], wk_sb[:, kc, :], xr(kc),
                        start=(kc == 0), stop=(kc == KC - 1),
                    )
                rope_evict(
                    psum, (kt_sb[0:64, cols], kt_sb[64:128, cols]), cols
                )
                # V (no rope) -> transpose to [tok, d] chunks
                psum = ps_pool.tile([P, TOKB], f32, name="ps")
                for kc in range(KC):
                    nc.tensor.matmul(
                        psum[:, :], wv_sb[:, kc, :], xr(kc),
                        start=(kc == 0), stop=(kc == KC - 1),
                    )
                vtmp = vtmp_pool.tile([P, TOKB], bf16, name="vtmp")
                nc.scalar.copy(vtmp[:, :], psum[:, :])
                for c in range(4):
                    vps = rs_pool.tile([P, P], bf16, name="vps")
                    nc.tensor.transpose(
                        vps[:, :], vtmp[:, P * c:P * (c + 1)], ident[:, :]
                    )
                    nc.scalar.copy(v_sb[4 * nb + c][:, :], vps[:, :])

        # ============ Phase B+D: attention + interleaved o_proj ============
        with ExitStack() as bctx:
            bec = bctx.enter_context
            wo_pool = bec(tc.tile_pool(name="wo_sb", bufs=1))
            cx_pool = bec(tc.tile_pool(name="cx_sb", bufs=2))
            e_pool = bec(tc.tile_pool(name="e_sb", bufs=6))
            acc_pool = bec(tc.tile_pool(name="acc_sb", bufs=3))
            rbc_pool = bec(tc.tile_pool(name="rbc_sb", bufs=2))
            ctx_out_pool = bec(tc.tile_pool(name="ctx_sb", bufs=2))
            o_out_pool = bec(tc.tile_pool(name="o_sb", bufs=3))

            wo_sb = wo_pool.tile([P, KC, NQ], bf16, name="wo")
            nc.sync.dma_start(out=wo_sb[:, :, :], in_=woT[:, :, :])

            cx_tiles = {}

            def load_cx(t):
                # ag_out rows 128r+d; SBUF chunk index fc = 4r+h matches the
                # natural global-head feature order of woT.
                cx = cx_pool.tile([P, N_CORES, NHL, TOKB], bf16, name="cx")
                nc.sync.dma_start(
                    out=cx[:, :, :, :],
                    in_=ag_out[t][:, :].rearrange(
                        "(r d) (h c) -> d r h c", r=N_CORES, h=NHL
                    ),
                )
                cx_tiles[t] = cx

            def oproj_chunk(t, ot):
                cols = slice(TOKB * t, TOKB * (t + 1))
                cx = cx_tiles[t]
                psum = op_pool.tile([P, TOKB], f32, name="ops")
                for fc in range(KC):
                    nc.tensor.matmul(
                        psum[:, :],
                        wo_sb[:, fc, P * ot:P * (ot + 1)],
                        cx[:, fc // NHL, fc % NHL, :],
                        start=(fc == 0), stop=(fc == KC - 1),
                    )
                ov = o_out_pool.tile([P, TOKB], f32, name="ov")
                nc.vector.tensor_copy(ov[:, :], psum[:, :])
                nc.sync.dma_start(
                    out=outT[P * ot:P * (ot + 1), cols], in_=ov[:, :]
                )

            for tb in range(NB):
                b, qb = tb // QBS, tb % QBS
                if tb >= 2:
                    load_cx(tb - 2)
                qcols = slice(S * b + TOKB * qb, S * b + TOKB * (qb + 1))
                ctxw = ctx_out_pool.tile([P, NHL * TOKB], bf16, name="ctxw")
                for h in range(NHL):
                    nkt = 4 * qb + 4
                    qh = q_sb[:, h, qcols]
                    acc = acc_pool.tile([P, TOKB], f32, name="acc")
                    ctxp = ctxp_pool.tile([P, TOKB], f32, name="ctxp")
                    es = [None] * nkt

                    def q_lo(kt):
                        """first valid q column of chunk kt (within block)"""
                        j = kt - 4 * qb
                        return 0 if j < 0 else P * j

                    def emit_score(kt):
                        lo = q_lo(kt)
                        w = TOKB - lo
                        sp = ps_pool.tile([P, TOKB], f32, name="ps")
                        nc.tensor.matmul(
                            sp[:, lo:],
                            kt_sb[:, S * b + P * kt:S * b + P * (kt + 1)],
                            qh[:, lo:],
                            start=True, stop=True,
                        )
                        e = e_pool.tile([P, TOKB], bf16, name="e")
                        nc.scalar.activation(
                            e[:, lo:], sp[:, lo:], Exp, scale=SCALE
                        )
                        j = kt - 4 * qb
                        if j >= 0:
                            nc.vector.tensor_mul(
                                e[:, lo:lo + P], e[:, lo:lo + P],
                                tri_sb[:, :],
                            )
                        if kt == 0:
                            nc.vector.tensor_copy(acc[:, lo:], e[:, lo:])
                        else:
                            nc.vector.tensor_add(
                                acc[:, lo:], acc[:, lo:], e[:, lo:]
                            )
                        es[kt] = e

                    def emit_ctx(kt):
                        lo = q_lo(kt)
                        nc.tensor.matmul(
                            ctxp[:, lo:],
                            v_sb[16 * b + kt][:, :],
                            es[kt][:, lo:],
                            start=(kt == 0), stop=(kt == nkt - 1),
                        )

                    # score-lookahead emission: s0 s1 c0 s2 c1 ... c(n-1)
                    emit_score(0)
                    for kt in range(1, nkt):
                        emit_score(kt)
                        emit_ctx(kt - 1)
                    emit_ctx(nkt - 1)

                    # denominator: ones[128,128]^T @ acc = broadcast rowsum
                    accb = e_pool.tile([P, TOKB], bf16, name="e")
                    nc.vector.tensor_copy(accb[:, :], acc[:, :])
                    rbp = rs_pool.tile([P, TOKB], f32, name="rbp")
                    nc.tensor.matmul(
                        rbp[:, :], ones_sq[:, :], accb[:, :],
                        start=True, stop=True,
                    )
                    rbc = rbc_pool.tile([P, TOKB], f32, name="rbc")
                    nc.vector.reciprocal(rbc[:, :], rbp[:, :])
                    nc.vector.tensor_mul(
                        ctxw[:, TOKB * h:TOKB * (h + 1)], ctxp[:, :],
                        rbc[:, :],
                    )
                    # interleave one o_proj chunk of block tb-2 per head
                    if tb >= 2:
                        oproj_chunk(tb - 2, h)
                nc.sync.dma_start(out=ag_in[tb][:, :], in_=ctxw[:, :])
                nc.gpsimd.collective_compute(
                    "AllGather",
                    mybir.AluOpType.bypass,
                    replica_groups=[list(range(N_CORES))],
                    ins=[ag_in[tb][:, :].opt()],
                    outs=[ag_out[tb][:, :].opt()],
                )

            for t in (NB - 2, NB - 1):
                load_cx(t)
                for ot in range(NHL):
                    oproj_chunk(t, ot)

    nc.finalize()
    return nc


def _host_prep(x, positions, w_q, w_k, w_v, w_o):
    bf = ml_dtypes.bfloat16

    def feat_major(w):
        # [F, HID] -> [128, KC, F] (partition = hid%128... hid = 128*kc + p)
        F = w.shape[0]
        return np.ascontiguousarray(
            w.T.reshape(KC, P, F).transpose(1, 0, 2)
        ).astype(bf)

    xT = feat_major(x.reshape(T, HID))          # [128, 32, T]

    half = D // 2
    inv_freq = 1.0 / (10000.0 ** (np.arange(half, dtype=np.float32) / half))
    freqs = np.outer(np.asarray(positions, np.float32), inv_freq)  # [S, 64]
    cosT1 = np.cos(freqs).T.astype(np.float32)  # [64, S]
    sinT1 = np.sin(freqs).T.astype(np.float32)
    cosT = np.ascontiguousarray(np.concatenate([cosT1] * B, axis=1))
    sinT = np.ascontiguousarray(np.concatenate([sinT1] * B, axis=1))

    dk = np.arange(P, dtype=np.int64)[:, None]
    dq = np.arange(P, dtype=np.int64)[None, :]
    maskT = np.ascontiguousarray((dk <= dq).astype(np.float32)).astype(bf)

    in_maps = []
    for c in range(N_CORES):
        in_maps.append({
            "xT": xT,
            "wqT": feat_major(w_q[NQ * c:NQ * (c + 1), :]),
            "wkT": feat_major(w_k[D * c:D * (c + 1), :]),
            "wvT": feat_major(w_v[D * c:D * (c + 1), :]),
            "woT": feat_major(w_o[NQ * c:NQ * (c + 1), :]),
            "cosT": cosT, "sinT": sinT, "maskT": maskT,
        })
    return in_maps


def _ensure_ntff_hook():
    """The agent image's antenv lacks axon_hooks; synthesize it so
    run_bass_kernel_spmd(trace=True) can capture NTFF profiles."""
    import sys
    import types
    try:
        from antenv.axon_hooks import get_axon_ntff_profile_hook  # noqa: F401
        return
    except ImportError:
        pass
    import antenv
    mod = types.ModuleType("antenv.axon_hooks")
    _h = [None]
    mod.set_axon_ntff_profile_hook = lambda h: _h.__setitem__(0, h)
    mod.get_axon_ntff_profile_hook = lambda: _h[0]
    sys.modules["antenv.axon_hooks"] = mod
    antenv.axon_hooks = mod
    try:
        from trn_agent_boot.trn_boot import _ntff_profile_via_ctypes
        mod.set_axon_ntff_profile_hook(
            _ntff_profile_via_ctypes("/opt/axon/libaxon_pjrt.so")
        )
    except Exception:
        pass


def kernel(x, positions, w_q, w_k, w_v, w_o):
    global _BUILT, LAST_RESULTS
    from concourse.bass_utils import run_bass_kernel_spmd

    x = np.asarray(x)
    positions = np.asarray(positions)
    w_q = np.asarray(w_q, np.float32)
    w_k = np.asarray(w_k, np.float32)
    w_v = np.asarray(w_v, np.float32)
    w_o = np.asarray(w_o, np.float32)

    if _BUILT is None:
        _BUILT = _build()
    nc = _BUILT

    in_maps = _host_prep(x, positions, w_q, w_k, w_v, w_o)
    trace = os.environ.get("BASS_KERNEL_TRACE", "0") == "1"
    if trace:
        _ensure_ntff_hook()
    res = run_bass_kernel_spmd(
        nc, in_maps, core_ids=list(range(N_CORES)), trace=trace
    )
    LAST_RESULTS = res

    out = np.empty((T, HID), np.float32)
    for c in range(N_CORES):
        out[:, NQ * c:NQ * (c + 1)] = np.asarray(res.results[c]["outT"]).T
    return out.reshape(B, S, HID)


# revision 14
# speedup vs baseline: 1.2611x; 1.0549x over previous
"""GQA attention layer (B=2, S=2048, HID=4096, 32 Q heads / 8 KV heads, RoPE,
causal) on 8 TRN2 NeuronCores.

Strategy (tensor-parallel over heads):
  - core c owns Q heads 4c..4c+3 and KV head c (one full GQA group).
  - host pre-transposes x and weights into [128, kc, feat] layouts so every
    input loads with a handful of large DMAs and every on-chip matmul
    contracts over the partition axis with no on-chip transposes of x.
  - Q/K/V are SBUF-resident: projections write Q^T/K^T (RoPE fused into the
    PSUM eviction on DVE) and V (TensorE-transposed) straight into
    persistent SBUF tiles; attention reads them with zero DMA.
  - the PE stream is kept dense end-to-end: batch-0 attention heads are
    interleaved with the batch-1 projection groups (so the AllGather chain
    starts ~half-way into the projection phase), and batch-1 attention
    heads are interleaved with o_proj chunks of already-gathered blocks.
  - attention is emitted score-lookahead (s0,s1,c0,s2,c1,...) so the PE
    FIFO never waits on the ACT exp; causal diagonal 128-chunks use
    narrowed q-ranges (exact triangle FLOPs) with a single 128x128
    triangular mask multiply; softmax denominator accumulates in bf16 and
    becomes one ones[128x128] matmul broadcast + ACT copy + DVE divide.
  - ctx^T is AllGathered per 512-token block (8 chunks); the host
    concatenates the 8 per-core 512-column o_proj output slices.
"""

import os

os.environ.setdefault("NEURON_RT_DBG_RDH_CC", "0")

import numpy as np
import ml_dtypes

B, S, HID = 2, 2048, 4096
NH, NKV, D = 32, 8, 128
T = B * S            # 4096 flattened tokens
NQ = 512             # per-core q features (4 heads x 128)
P = 128
TOKB = 512           # token block (matmul moving free dim)
NB = T // TOKB       # 8 token blocks
KC = HID // P        # 32 contraction chunks for projections
QBS = S // TOKB      # 4 q blocks per batch
KTS = S // P         # 16 k chunks per batch
NHL = 4              # local Q heads per core
SCALE = 1.0 / float(np.sqrt(np.float32(D)))
N_CORES = 8

_BUILT = None
LAST_RESULTS = None


def _build():
    from contextlib import ExitStack

    import concourse.tile as tile
    from concourse import bacc, mybir

    f32 = mybir.dt.float32
    bf16 = mybir.dt.bfloat16
    Exp = mybir.ActivationFunctionType.Exp

    nc = bacc.Bacc(
        "TRN2",
        target_bir_lowering=False,
        debug=False,
        num_devices=N_CORES,
    )

    xT = nc.declare_dram_parameter("xT", [P, KC, T], bf16, isOutput=False)
    wqT = nc.declare_dram_parameter("wqT", [P, KC, NQ], bf16, isOutput=False)
    wkT = nc.declare_dram_parameter("wkT", [P, KC, D], bf16, isOutput=False)
    wvT = nc.declare_dram_parameter("wvT", [P, KC, D], bf16, isOutput=False)
    woT = nc.declare_dram_parameter("woT", [P, KC, NQ], bf16, isOutput=False)
    cosT = nc.declare_dram_parameter("cosT", [64, T], f32, isOutput=False)
    sinT = nc.declare_dram_parameter("sinT", [64, T], f32, isOutput=False)
    maskT = nc.declare_dram_parameter("maskT", [P, P], bf16, isOutput=False)
    outT = nc.declare_dram_parameter("outT", [NQ, T], f32, isOutput=True)

    XC = 8               # kc chunks per x DMA
    NXD = KC // XC       # 4 x DMAs per token block

    with tile.TileContext(nc) as tc, ExitStack() as gctx:
        ec = gctx.enter_context
        # ---- global pools (whole-kernel lifetime) ----
        dram = ec(tc.tile_pool(name="dram", bufs=1, space="DRAM"))
        const_pool = ec(tc.tile_pool(name="const_sb", bufs=1))
        qkv_pool = ec(tc.tile_pool(name="qkv_sb", bufs=1))
        # PSUM budget (8 banks): ps 3 + ctxp 2 + rsp {vps,rbp} 2 + opp 1
        ps_pool = ec(tc.tile_pool(name="ps", bufs=3, space="PSUM"))
        ctxp_pool = ec(tc.tile_pool(name="ctxp", bufs=2, space="PSUM"))
        rs_pool = ec(tc.tile_pool(name="rsp", bufs=1, space="PSUM"))
        op_pool = ec(tc.tile_pool(name="opp", bufs=1, space="PSUM"))
        # attention working pools (live through both halves)
        e_pool = ec(tc.tile_pool(name="e_sb", bufs=6))
        acc_pool = ec(tc.tile_pool(name="acc_sb", bufs=3))
        rbc_pool = ec(tc.tile_pool(name="rbc_sb", bufs=2))
        ctx_out_pool = ec(tc.tile_pool(name="ctx_sb", bufs=2))

        # token-chunked AllGather buffers (0.5 MB per rank per chunk)
        ag_in = [dram.tile([P, NHL * TOKB], bf16, name=f"ag_in{t}")
                 for t in range(NB)]
        ag_out = [
            dram.tile([N_CORES * P, NHL * TOKB], bf16, addr_space="Shared",
                      name=f"ag_out{t}")
            for t in range(NB)
        ]

        ones_sq = const_pool.tile([P, P], bf16, name="ones_sq")
        nc.vector.memset(ones_sq[:, :], 1.0)
        ident = const_pool.tile([P, P], bf16, name="ident")
        from concourse.masks import make_identity
        make_identity(nc, ident[:, :])
        tri_sb = const_pool.tile([P, P], bf16, name="tri_sb")
        nc.sync.dma_start(out=tri_sb[:, :], in_=maskT[:, :])

        # persistent Q^T / K^T / V tiles (SBUF-resident between phases)
        q_sb = qkv_pool.tile([P, NHL, T], bf16, name="q_sb")
        kt_sb = qkv_pool.tile([P, T], bf16, name="kt_sb")
        v_sb = [qkv_pool.tile([P, P], bf16, name=f"v_sb{g}")
                for g in range(T // P)]  # 32 chunks

        # ---------------- attention head / AG emitters ----------------
        def attn_head(tb, h, ctxw):
            b, qb = tb // QBS, tb % QBS
            qcols = slice(S * b + TOKB * qb, S * b + TOKB * (qb + 1))
            nkt = 4 * qb + 4
            qh = q_sb[:, h, qcols]
            acc = acc_pool.tile([P, TOKB], bf16, name="acc")
            ctxp = ctxp_pool.tile([P, TOKB], f32, name="ctxp")
            es = [None] * nkt

            def q_lo(kt):
                j = kt - 4 * qb
                return 0 if j < 0 else P * j

            def emit_score(kt):
                lo = q_lo(kt)
                sp = ps_pool.tile([P, TOKB], f32, name="ps")
                nc.tensor.matmul(
                    sp[:, lo:],
                    kt_sb[:, S * b + P * kt:S * b + P * (kt + 1)],
                    qh[:, lo:],
                    start=True, stop=True,
                )
                e = e_pool.tile([P, TOKB], bf16, name="e")
                nc.scalar.activation(e[:, lo:], sp[:, lo:], Exp, scale=SCALE)
                if kt - 4 * qb >= 0:
                    nc.vector.tensor_mul(
                        e[:, lo:lo + P], e[:, lo:lo + P], tri_sb[:, :]
                    )
                if kt == 0:
                    nc.vector.tensor_copy(acc[:, lo:], e[:, lo:])
                else:
                    nc.vector.tensor_add(acc[:, lo:], acc[:, lo:], e[:, lo:])
                es[kt] = e

            def emit_ctx(kt):
                lo = q_lo(kt)
                nc.tensor.matmul(
                    ctxp[:, lo:],
                    v_sb[16 * b + kt][:, :],
                    es[kt][:, lo:],
                    start=(kt == 0), stop=(kt == nkt - 1),
                )

            # score-lookahead emission: s0 s1 c0 s2 c1 ... c(n-1)
            emit_score(0)
            for kt in range(1, nkt):
                emit_score(kt)
                emit_ctx(kt - 1)
            emit_ctx(nkt - 1)

            # denominator: ones[128,128]^T @ acc = broadcast rowsum
            rbp = rs_pool.tile([P, TOKB], f32, name="rbp")
            nc.tensor.matmul(
                rbp[:, :], ones_sq[:, :], acc[:, :], start=True, stop=True
            )
            rbc = rbc_pool.tile([P, TOKB], f32, name="rbc")
            nc.vector.reciprocal(rbc[:, :], rbp[:, :])
            nc.vector.tensor_mul(
                ctxw[:, TOKB * h:TOKB * (h + 1)], ctxp[:, :], rbc[:, :]
            )

        def emit_ag(tb, ctxw):
            nc.sync.dma_start(out=ag_in[tb][:, :], in_=ctxw[:, :])
            nc.gpsimd.collective_compute(
                "AllGather",
                mybir.AluOpType.bypass,
                replica_groups=[list(range(N_CORES))],
                ins=[ag_in[tb][:, :].opt()],
                outs=[ag_out[tb][:, :].opt()],
            )

        # ================= Projections + batch-0 attention =================
        with ExitStack() as actx:
            aec = actx.enter_context
            w_pool = aec(tc.tile_pool(name="w_sb", bufs=1))
            xt_pool = aec(tc.tile_pool(name="xt_sb", bufs=5))
            rope_pool = aec(tc.tile_pool(name="rope_sb", bufs=1))
            rtmp_pool = aec(tc.tile_pool(name="rtmp_sb", bufs=2))
            vtmp_pool = aec(tc.tile_pool(name="vtmp_sb", bufs=2))

            wq_sb = w_pool.tile([P, KC, NQ], bf16, name="wq")
            nc.sync.dma_start(out=wq_sb[:, :, :], in_=wqT[:, :, :])
            wk_sb = w_pool.tile([P, KC, D], bf16, name="wk")
            nc.sync.dma_start(out=wk_sb[:, :, :], in_=wkT[:, :, :])
            wv_sb = w_pool.tile([P, KC, D], bf16, name="wv")
            nc.sync.dma_start(out=wv_sb[:, :, :], in_=wvT[:, :, :])
            cos_sb = rope_pool.tile([64, T], f32, name="cos_sb")
            nc.sync.dma_start(out=cos_sb[:, :], in_=cosT[:, :])
            sin_sb = rope_pool.tile([64, T], f32, name="sin_sb")
            nc.sync.dma_start(out=sin_sb[:, :], in_=sinT[:, :])

            def rope_evict(psum, dest, cols):
                """psum [128(d), 512(tok)] f32 -> RoPE -> dest bf16 slices."""
                c_ap = cos_sb[:, cols]
                s_ap = sin_sb[:, cols]
                p0 = psum[0:64, :]
                p1 = psum[64:128, :]
                t0 = rtmp_pool.tile([64, TOKB], f32, name="t0")
                t1 = rtmp_pool.tile([64, TOKB], f32, name="t1")
                nc.vector.tensor_mul(t0[:, :], p0, c_ap)
                nc.vector.tensor_mul(t1[:, :], p1, s_ap)
                nc.vector.tensor_sub(dest[0], t0[:, :], t1[:, :])
                t2 = rtmp_pool.tile([64, TOKB], f32, name="t2")
                t3 = rtmp_pool.tile([64, TOKB], f32, name="t3")
                nc.vector.tensor_mul(t2[:, :], p0, s_ap)
                nc.vector.tensor_mul(t3[:, :], p1, c_ap)
                nc.vector.tensor_add(dest[1], t2[:, :], t3[:, :])

            def proj_thunks(nb):
                """6 emission thunks for token block nb; thunk 0 also emits
                the x DMAs."""
                cols = slice(TOKB * nb, TOKB * (nb + 1))
                xts = []

                def load_x():
                    for xd in range(NXD):
                        xt_t = xt_pool.tile([P, XC, TOKB], bf16, name="xt")
                        nc.sync.dma_start(
                            out=xt_t[:, :, :],
                            in_=xT[:, XC * xd:XC * (xd + 1), cols],
                        )
                        xts.append(xt_t)

                def xr(kc):
                    return xts[kc // XC][:, kc % XC, :]

                def q_group(m):
                    psum = ps_pool.tile([P, TOKB], f32, name="ps")
                    for kc in range(KC):
                        nc.tensor.matmul(
                            psum[:, :],
                            wq_sb[:, kc, P * m:P * (m + 1)],
                            xr(kc),
                            start=(kc == 0), stop=(kc == KC - 1),
                        )
                    rope_evict(
                        psum,
                        (q_sb[0:64, m, cols], q_sb[64:128, m, cols]),
                        cols,
                    )

                def k_group():
                    psum = ps_pool.tile([P, TOKB], f32, name="ps")
                    for kc in range(KC):
                        nc.tensor.matmul(
                            psum[:, :], wk_sb[:, kc, :], xr(kc),
                            start=(kc == 0), stop=(kc == KC - 1),
                        )
                    rope_evict(
                        psum, (kt_sb[0:64, cols], kt_sb[64:128, cols]), cols
                    )

                def v_group():
                    psum = ps_pool.tile([P, TOKB], f32, name="ps")
                    for kc in range(KC):
                        nc.tensor.matmul(
                            psum[:, :], wv_sb[:, kc, :], xr(kc),
                            start=(kc == 0), stop=(kc == KC - 1),
                        )
                    vtmp = vtmp_pool.tile([P, TOKB], bf16, name="vtmp")
                    nc.scalar.copy(vtmp[:, :], psum[:, :])
                    for c in range(4):
                        vps = rs_pool.tile([P, P], bf16, name="vps")
                        nc.tensor.transpose(
                            vps[:, :], vtmp[:, P * c:P * (c + 1)], ident[:, :]
                        )
                        nc.scalar.copy(v_sb[4 * nb + c][:, :], vps[:, :])

                def first():
                    load_x()
                    q_group(0)

                return ([first] + [lambda m=m: q_group(m) for m in (1, 2, 3)]
                        + [k_group, v_group])

            # part 1: projections for batch 0 (blocks 0..3), dense
            for nb in range(4):
                for th in proj_thunks(nb):
                    th()

            # part 2: batch-0 attention interleaved with batch-1 projections
            filler = []
            for nb in range(4, 8):
                filler.extend(proj_thunks(nb))
            pops = [2] * 8 + [1] * 8
            slot = 0
            for tb in range(4):
                ctxw = ctx_out_pool.tile([P, NHL * TOKB], bf16, name="ctxw")
                for h in range(NHL):
                    attn_head(tb, h, ctxw)
                    for _ in range(pops[slot]):
                        if filler:
                            filler.pop(0)()
                    slot += 1
                emit_ag(tb, ctxw)
            while filler:
                filler.pop(0)()

        # ============ batch-1 attention + interleaved o_proj ============
        with ExitStack() as bctx:
            bec = bctx.enter_context
            wo_pool = bec(tc.tile_pool(name="wo_sb", bufs=1))
            cx_pool = bec(tc.tile_pool(name="cx_sb", bufs=2))
            o_out_pool = bec(tc.tile_pool(name="o_sb", bufs=3))

            wo_sb = wo_pool.tile([P, KC, NQ], bf16, name="wo")
            nc.sync.dma_start(out=wo_sb[:, :, :], in_=woT[:, :, :])

            cx_tiles = {}

            def load_cx(t):
                # ag_out rows 128r+d; SBUF chunk index fc = 4r+h matches the
                # natural global-head feature order of woT.
                cx = cx_pool.tile([P, N_CORES, NHL, TOKB], bf16, name="cx")
                nc.sync.dma_start(
                    out=cx[:, :, :, :],
                    in_=ag_out[t][:, :].rearrange(
                        "(r d) (h c) -> d r h c", r=N_CORES, h=NHL
                    ),
                )
                cx_tiles[t] = cx

            def oproj_chunk(t, ot):
                cols = slice(TOKB * t, TOKB * (t + 1))
                cx = cx_tiles[t]
                psum = op_pool.tile([P, TOKB], f32, name="ops")
                for fc in range(KC):
                    nc.tensor.matmul(
                        psum[:, :],
                        wo_sb[:, fc, P * ot:P * (ot + 1)],
                        cx[:, fc // NHL, fc % NHL, :],
                        start=(fc == 0), stop=(fc == KC - 1),
                    )
                ov = o_out_pool.tile([P, TOKB], f32, name="ov")
                nc.vector.tensor_copy(ov[:, :], psum[:, :])
                nc.sync.dma_start(
                    out=outT[P * ot:P * (ot + 1), cols], in_=ov[:, :]
                )

            load_cx(0)
            load_cx(1)
            opq = [(t, ot) for t in range(NB - 2) for ot in range(NHL)]
            pops2 = [1, 1, 2, 2, 2, 2, 2, 2, 2, 2, 1, 1, 1, 1, 1, 1]
            slot = 0
            for tb in range(4, NB):
                ctxw = ctx_out_pool.tile([P, NHL * TOKB], bf16, name="ctxw")
                for h in range(NHL):
                    attn_head(tb, h, ctxw)
                    for _ in range(pops2[slot]):
                        if opq:
                            t, ot = opq.pop(0)
                            if t not in cx_tiles:
                                load_cx(t)
                            oproj_chunk(t, ot)
                    slot += 1
                emit_ag(tb, ctxw)
            while opq:
                t, ot = opq.pop(0)
                if t not in cx_tiles:
                    load_cx(t)
                oproj_chunk(t, ot)

            for t in (NB - 2, NB - 1):
                load_cx(t)
                for ot in range(NHL):
                    oproj_chunk(t, ot)

    nc.finalize()
    return nc


def _host_prep(x, positions, w_q, w_k, w_v, w_o):
    bf = ml_dtypes.bfloat16

    def feat_major(w):
        # [F, HID] -> [128, KC, F]  (hid = 128*kc + p)
        F = w.shape[0]
        return np.ascontiguousarray(
            w.T.reshape(KC, P, F).transpose(1, 0, 2)
        ).astype(bf)

    xT = feat_major(x.reshape(T, HID))          # [128, 32, T]

    half = D // 2
    inv_freq = 1.0 / (10000.0 ** (np.arange(half, dtype=np.float32) / half))
    freqs = np.outer(np.asarray(positions, np.float32), inv_freq)  # [S, 64]
    cosT1 = np.cos(freqs).T.astype(np.float32)  # [64, S]
    sinT1 = np.sin(freqs).T.astype(np.float32)
    cosT = np.ascontiguousarray(np.concatenate([cosT1] * B, axis=1))
    sinT = np.ascontiguousarray(np.concatenate([sinT1] * B, axis=1))

    dk = np.arange(P, dtype=np.int64)[:, None]
    dq = np.arange(P, dtype=np.int64)[None, :]
    maskT = np.ascontiguousarray((dk <= dq).astype(np.float32)).astype(bf)

    in_maps = []
    for c in range(N_CORES):
        in_maps.append({
            "xT": xT,
            "wqT": feat_major(w_q[NQ * c:NQ * (c + 1), :]),
            "wkT": feat_major(w_k[D * c:D * (c + 1), :]),
            "wvT": feat_major(w_v[D * c:D * (c + 1), :]),
            "woT": feat_major(w_o[NQ * c:NQ * (c + 1), :]),
            "cosT": cosT, "sinT": sinT, "maskT": maskT,
        })
    return in_maps


def _ensure_ntff_hook():
    """The agent image's antenv lacks axon_hooks; synthesize it so
    run_bass_kernel_spmd(trace=True) can capture NTFF profiles."""
    import sys
    import types
    try:
        from antenv.axon_hooks import get_axon_ntff_profile_hook  # noqa: F401
        return
    except ImportError:
        pass
    import antenv
    mod = types.ModuleType("antenv.axon_hooks")
    _h = [None]
    mod.set_axon_ntff_profile_hook = lambda h: _h.__setitem__(0, h)
    mod.get_axon_ntff_profile_hook = lambda: _h[0]
    sys.modules["antenv.axon_hooks"] = mod
    antenv.axon_hooks = mod
    try:
        from trn_agent_boot.trn_boot import _ntff_profile_via_ctypes
        mod.set_axon_ntff_profile_hook(
            _ntff_profile_via_ctypes("/opt/axon/libaxon_pjrt.so")
        )
    except Exception:
        pass


def kernel(x, positions, w_q, w_k, w_v, w_o):
    global _BUILT, LAST_RESULTS
    from concourse.bass_utils import run_bass_kernel_spmd

    x = np.asarray(x)
    positions = np.asarray(positions)
    w_q = np.asarray(w_q, np.float32)
    w_k = np.asarray(w_k, np.float32)
    w_v = np.asarray(w_v, np.float32)
    w_o = np.asarray(w_o, np.float32)

    if _BUILT is None:
        _BUILT = _build()
    nc = _BUILT

    in_maps = _host_prep(x, positions, w_q, w_k, w_v, w_o)
    trace = os.environ.get("BASS_KERNEL_TRACE", "0") == "1"
    if trace:
        _ensure_ntff_hook()
    res = run_bass_kernel_spmd(
        nc, in_maps, core_ids=list(range(N_CORES)), trace=trace
    )
    LAST_RESULTS = res

    out = np.empty((T, HID), np.float32)
    for c in range(N_CORES):
        out[:, NQ * c:NQ * (c + 1)] = np.asarray(res.results[c]["outT"]).T
    return out.reshape(B, S, HID)
